# revision 1
# baseline (speedup 1.0000x reference)
"""Trainium2 Bass kernel for nn_BlockUngrouper.

Problem: out[b, n, :] = block_features[b, g, k, :] where g is the block whose
one-hot claims token n and k is n's rank within that block (cumsum of the
one-hot along n).  The input distribution (per-sample permutation partition)
guarantees each token is claimed by exactly one block and ranks < 128, so per
batch this is a row-permutation gather with
    flat_idx[n] = 128 * g(n) + rank(n).

Sharding: data-parallel over the batch dim, 2 batches per NeuronCore x 8.

Per-core program (all index arithmetic exact in fp32):
  1. onehot [N, 128] -> SBUF f32 chunks (HWDGE), ACT-engine copy casts to
     bf16 in layout [token-in-tile, (tile, g)].
  2. counts[g, t]: per 128-token tile, PE matmul lhsT=OH rhs=ones.
  3. incl/pex[g, t]: scan over tiles (DVE tensor_tensor_scan), add
     128*g - 1 + b*N (gmat), PE-transpose to [t, g], flatten 16-tile slices
     to partition-0 rows (small HWDGE SBUF->SBUF DMA).
  4. per 4-tile PSUM group: broadcast-add prefix via K=1 matmul (start),
     then 4 upper-triangular matmuls add the within-tile inclusive cumsum;
     DVE scalar_tensor_tensor (PSUM x onehot, accum_out over g) selects each
     token's entry -> flat_mat[p, t] = feat row index (b*N included).
  5. idx shuffle: 8 selector matmuls (Rall) reshape flat_mat [p, t] into the
     dma_gather index layout (idx j at partition j%16, col j//16, replicated
     x8 across partition groups), one DVE copy casts PSUM->int16 with an
     interleaving access pattern.
  6. per 16-tile chunk (2048 tokens): ONE bulk SWDGE dma_gather pulls 2048 x
     1KB feature rows into SBUF (2 MB), then one HWDGE store DMA writes them
     to out.  8 gathers + 8 stores per batch vs 256 indirect DMAs in v7 --
     SWDGE descriptor-generation cost drops from ~256us to ~27us per core.
"""

from contextlib import ExitStack

import numpy as np

import concourse.bass as bass
import concourse.bacc as bacc
import concourse.mybir as mybir
import concourse.tile as tile
from concourse import bass_utils
from concourse.masks import make_identity, make_upper_triangular
from concourse import library_config

P = 128  # partitions = tokens per tile = G (blocks) = NG_MAX
KERNEL_VERSION = 9  # bump on every meaningful kernel change (NEFF-cache buster)
N_CORES = 8
B_FULL = 16  # full batch dim
N_TOK = 16384  # tokens per batch
D_FEAT = 256  # feature dim
NB = B_FULL // N_CORES  # batches per core

FP32 = mybir.dt.float32
BF16 = mybir.dt.bfloat16
I16 = mybir.dt.int16
I32 = mybir.dt.int32


def build_nc(NB: int, N: int, D: int, CT: int = 8, GRP: int = 4, LC: int = 16,
             STG_BUFS: int = 2, IDX_BUFS: int = 2, REPS: int = 1,
             DYN_LOOP: int = 0, MODE: str = "full", SCRATCH: int = 65536,
             TIMING: bool = False, FEAT_ROWS: int = 0, SP: int = 1,
             QN: int = 1, LAYOUT: str = "pk"):
    """Build the per-core bass program.

    NB: batches per core; N: tokens per batch; D: feature dim.
    CT: tiles per gather/store chunk; GRP: tiles per PSUM group; LC: tiles
    per onehot load chunk.
    TIMING: features/out become Internal DRAM tensors (no per-call host
    transfer; gather addresses depend only on onehot, values don't matter),
    and a tiny dummy output keeps the NEFF valid.  Timing-only builds.
    """
    T = N // P  # token tiles per batch
    assert T * P == N
    GRP = min(GRP, T)
    CT = min(CT, T)
    LC = min(LC, T)
    assert T % GRP == 0 and T % CT == 0 and T % LC == 0
    NI = CT * P  # tokens (gather indices) per chunk
    add = mybir.AluOpType.add
    mult = mybir.AluOpType.mult
    bypass = mybir.AluOpType.bypass

    nc = bacc.Bacc("TRN2", target_bir_lowering=False, debug=False,
                   dynamic_dma_scratch_size=SCRATCH, num_swdge_queues=QN)

    io_kind = "Internal" if TIMING else None
    FEAT_ROWS = FEAT_ROWS or N  # feat rows per batch (128 * G in general)
    feat = nc.dram_tensor("block_features", [NB * FEAT_ROWS, D], FP32,
                          kind=io_kind or "ExternalInput")
    oh = nc.dram_tensor("block_onehot", [NB, N, P], FP32, kind="ExternalInput")
    out = nc.dram_tensor("out", [NB, N, D], FP32, kind=io_kind or "ExternalOutput")
    if TIMING:
        nc.dram_tensor("dummy_out", [1, 8], FP32, kind="ExternalOutput")
    # The PJRT NEFF cache keys on the HLO alone (the embedded bass program
    # does not enter the hash), so distinct kernel versions collide.  A dummy
    # input whose shape encodes a version nonce forces a distinct hash.
    import zlib as _zlib
    _nonce = (
        _zlib.crc32(
            f"v{KERNEL_VERSION}-{NB}-{N}-{D}-{CT}-{GRP}-{LC}-{STG_BUFS}-{IDX_BUFS}-{REPS}-{DYN_LOOP}-{MODE}-{SCRATCH}-{TIMING}-{FEAT_ROWS}-{SP}-{QN}-{LAYOUT}".encode()
        )
        % 4093
        + 1
    )
    nc.dram_tensor("version_tag", [1, _nonce], FP32, kind="ExternalInput")

    with tile.TileContext(nc) as tc, ExitStack() as ctx:
        cpool = ctx.enter_context(tc.tile_pool(name="const", bufs=1))
        ohpool = ctx.enter_context(tc.tile_pool(name="ohp", bufs=2))
        ldpool = ctx.enter_context(tc.tile_pool(name="ld", bufs=2))
        wpool = ctx.enter_context(tc.tile_pool(name="work", bufs=2))
        ppool = ctx.enter_context(tc.tile_pool(name="psum", bufs=2, space="PSUM"))
        pspool = ctx.enter_context(tc.tile_pool(name="psumsm", bufs=2, space="PSUM"))
        spool = ctx.enter_context(tc.tile_pool(name="stage", bufs=STG_BUFS))
        fpool = ctx.enter_context(tc.tile_pool(name="flat", bufs=2))
        xpool = ctx.enter_context(tc.tile_pool(name="xrep", bufs=IDX_BUFS))

        # --- constants ---
        triu = cpool.tile([P, P], BF16)  # triu[k, m] = 1 iff k <= m
        make_upper_triangular(nc, triu[:], val=1.0, diag=True)
        ident = cpool.tile([P, P], FP32)
        make_identity(nc, ident[:])
        ones_col = cpool.tile([P, 1], BF16)
        nc.gpsimd.memset(ones_col[:], 1.0)
        ones_row = cpool.tile([1, P], FP32)
        nc.gpsimd.memset(ones_row[:], 1.0)
        # gmat_b[g, t] = 128*g - 1 + b*N (constant along t)
        gmats = []
        for b in range(NB):
            gi = cpool.tile([P, T], I32, tag=f"gi{b}")
            nc.gpsimd.iota(gi[:], pattern=[[0, T]], base=b * FEAT_ROWS - 1,
                           channel_multiplier=P)
            gf = cpool.tile([P, T], FP32, tag=f"gmat{b}")
            nc.vector.tensor_copy(gf[:], gi[:])
            gmats.append(gf)
        # Rall[p, m*128 + p'] = 1 iff p == 16*m + p'%16  (selector blocks for
        # the dma_gather index shuffle: block m broadcasts flat_mat rows
        # 16m..16m+15 to every 16-partition group)
        e_i = cpool.tile([P, 8 * P], I32)
        nc.gpsimd.iota(e_i[:], pattern=[[16, 8], [0, 8], [1, 16]], base=0,
                       channel_multiplier=0)
        p_i = cpool.tile([P, 8 * P], I32)
        nc.gpsimd.iota(p_i[:], pattern=[[0, 8 * P]], base=0,
                       channel_multiplier=1)
        rall = cpool.tile([P, 8 * P], FP32)
        nc.vector.tensor_tensor(out=rall[:], in0=e_i[:], in1=p_i[:],
                                op=mybir.AluOpType.is_equal)
        # Rall2[p, k*128 + q'] = 1 iff p == 8*(q'%16) + k  (pk-layout selector:
        # slot j of a 1024-token chunk maps to token base + 8*(j%128) + j//128,
        # so each partition's store run is 8 rows = 8KB contiguous)
        e2_i = cpool.tile([P, 8 * P], I32, tag="e2i")
        nc.gpsimd.iota(e2_i[:], pattern=[[1, 8], [0, 8], [8, 16]], base=0,
                       channel_multiplier=0)
        rall2 = cpool.tile([P, 8 * P], FP32, tag="rall2")
        nc.vector.tensor_tensor(out=rall2[:], in0=e2_i[:], in1=p_i[:],
                                op=mybir.AluOpType.is_equal)
        stg_c = None
        if MODE == "sonly":
            stg_c = cpool.tile([P, CT * D], FP32, tag="stgc")
            nc.gpsimd.memset(stg_c[:], 1.0)
        fiota = []
        if MODE in ("gather", "gonly", "sonly", "lonly", "glonly", "igonly", "sconly", "gonly2k"):
            for b in range(NB):
                fi_i = cpool.tile([P, T], I32, tag=f"fi{b}")
                nc.gpsimd.iota(fi_i[:], pattern=[[P, T]], base=b * FEAT_ROWS,
                               channel_multiplier=1)
                ff = cpool.tile([P, T], FP32, tag=f"ff{b}")
                nc.vector.tensor_copy(ff[:], fi_i[:])
                fiota.append(ff)

        import contextlib
        loop_cm = tc.For_i(0, DYN_LOOP, 1) if DYN_LOOP else contextlib.nullcontext()
        with loop_cm:
          for rep in range(REPS):
            for b in range(NB):
                oh_src = oh.ap()[b].rearrange("(t p) g -> p t g", p=P)
                out_dst = out.ap()[b].rearrange("(t p) d -> p t d", p=P)

                if MODE == "gonly2k":
                    # dma_gather with 2KB elements (feat viewed [rows/2, 512]):
                    # same bytes, half the descriptors (iota only).
                    xr2 = xpool.tile([P, 4 * T], I16, tag="xr2")
                    fi2 = wpool.tile([P, 4 * T], I32, tag="fi2")
                    # near-sequential in-bounds rows; exact values don't matter
                    # for this throughput probe (TIMING gonly only)
                    nc.gpsimd.iota(fi2[:], pattern=[[15, 4 * T]], base=b * FEAT_ROWS // 2,
                                   channel_multiplier=1)
                    nc.vector.tensor_copy(xr2[:], fi2[:])
                    for c in range(T // CT):
                        stg = spool.tile([P, CT * D], FP32, tag="stg")
                        nc.gpsimd.dma_gather(
                            stg[:].rearrange("p (t d) -> p t d", d=2 * D),
                            feat.ap().rearrange("(r two) d -> r (two d)", two=2),
                            xr2[:, c * CT * 4: (c + 1) * CT * 4],
                            NI // 2,
                            NI // 2,
                            2 * D,
                            single_packet=bool(SP),
                        )
                    continue
                if MODE == "igonly":
                    # Bulk indirect gather: one op per CT-tile chunk with a
                    # [128, CT] offset block (firmware SWDGE descriptors).
                    idx_i = wpool.tile([P, T], I32, tag="idxi")
                    nc.vector.tensor_copy(idx_i[:], fiota[b][:])
                    for c in range(T // CT):
                        stg = spool.tile([P, CT * D], FP32, tag="stg")
                        nc.gpsimd.indirect_dma_start(
                            out=stg[:].rearrange("p (t d) -> p t d", d=D),
                            out_offset=None,
                            in_=feat.ap(),
                            in_offset=bass.IndirectOffsetOnAxis(
                                ap=idx_i[:, c * CT: (c + 1) * CT], axis=0
                            ),
                        )
                    continue
                if MODE == "glonly":
                    # Sequential 1KB-descriptor SWDGE reads of feat.
                    for c in range(T // CT):
                        stg = spool.tile([P, CT * D], FP32, tag="stg")
                        nc.gpsimd.dma_start(
                            out=stg[:].rearrange("p (t d) -> p t d", d=D),
                            in_=feat.ap()[b * FEAT_ROWS: (b + 1) * FEAT_ROWS]
                            .rearrange("(t p) d -> p t d", p=P)[:, c * CT: (c + 1) * CT, :],
                        )
                    continue
                if MODE == "lonly":
                    # Sequential 1KB-descriptor HWDGE reads of feat.
                    for c in range(T // CT):
                        stg = spool.tile([P, CT * D], FP32, tag="stg")
                        nc.sync.dma_start(
                            out=stg[:].rearrange("p (t d) -> p t d", d=D),
                            in_=feat.ap()[b * FEAT_ROWS: (b + 1) * FEAT_ROWS]
                            .rearrange("(t p) d -> p t d", p=P)[:, c * CT: (c + 1) * CT, :],
                        )
                    continue
                if MODE == "sonly":
                    # Sequential HWDGE writes to out (desc size per LAYOUT).
                    for c in range(T // CT):
                        if LAYOUT == "pk":
                            nc.sync.dma_start(
                                out=out.ap()[b][c * NI: (c + 1) * NI, :]
                                .rearrange("(r k) d -> r k d", r=P),
                                in_=stg_c[:],
                            )
                        else:
                            nc.sync.dma_start(
                                out=out_dst[:, c * CT: (c + 1) * CT, :], in_=stg_c[:]
                            )
                    continue
                if MODE in ("gather", "gonly", "sconly"):
                    # Diagnostic: iota indices (from constants) through the
                    # real idx-shuffle machinery, then gathers + stores only.
                    flat_mat = fiota[b]
                else:
                    flat_mat = wpool.tile([P, T], FP32, tag="flatmat")
                    oh_sb = ohpool.tile([P, T * P], BF16, tag="oh")
                    countsT_ps = pspool.tile([P, T], FP32, tag="counts")
                    incl = wpool.tile([P, T], FP32, tag="incl")  # [g, t] incl
                    pex_adj = wpool.tile([P, T], FP32, tag="pexadj")
                    padjT = wpool.tile([T, P], FP32, tag="padjT")  # [t, g]

                    # --- load + cast + counts + scan per LC chunk ---
                    for lc in range(T // LC):
                        lc0, lc1 = lc * LC, (lc + 1) * LC
                        ld = ldpool.tile([P, LC * P], FP32, tag="ld")
                        nc.scalar.dma_start(out=ld[:], in_=oh_src[:, lc0:lc1, :])
                        nc.scalar.copy(oh_sb[:, lc0 * P: lc1 * P], ld[:])
                        for t in range(lc0, lc1):
                            nc.tensor.matmul(
                                out=countsT_ps[:, t: t + 1],
                                lhsT=oh_sb[:, t * P: (t + 1) * P],
                                rhs=ones_col[:],
                                start=True,
                                stop=True,
                            )
                        nc.vector.tensor_tensor_scan(
                            out=incl[:, lc0:lc1],
                            data0=countsT_ps[:, lc0:lc1],
                            data1=gmats[b][:, 0:LC],
                            initial=(0.0 if lc == 0 else incl[:, lc0 - 1: lc0]),
                            op0=add,
                            op1=bypass,
                        )
                    # --- prefix: pex_adj = incl - counts + gmat; transpose;
                    #     flatten slices to partition-0 rows ---
                    nc.vector.tensor_tensor(
                        out=pex_adj[:], in0=incl[:], in1=countsT_ps[:],
                        op=mybir.AluOpType.subtract,
                    )
                    nc.vector.tensor_tensor(
                        out=pex_adj[:], in0=pex_adj[:], in1=gmats[b][:],
                        op=add,
                    )
                    padjT_ps = pspool.tile([T, P], FP32, tag="padjT_ps")
                    nc.tensor.transpose(
                        out=padjT_ps[:], in_=pex_adj[:], identity=ident[:]
                    )
                    nc.vector.tensor_copy(padjT[:], padjT_ps[:])
                    SL = 8  # tiles per flatten slice
                    for s in range(T // SL):
                        flat_row = fpool.tile([1, SL * P], FP32, tag="flatrow")
                        nc.scalar.dma_start(
                            out=flat_row[:],
                            in_=padjT[s * SL: (s + 1) * SL, :],
                        )
                        # --- groups: prefix bcast + within-tile cumsum + select
                        for grp in range(s * SL // GRP, (s + 1) * SL // GRP):
                            g_in_s = grp - s * SL // GRP
                            grp_ps = ppool.tile([P, GRP * P], FP32, tag="grp")
                            nc.tensor.matmul(
                                out=grp_ps[:],
                                lhsT=ones_row[:],
                                rhs=flat_row[0:1, g_in_s * GRP * P:
                                             (g_in_s + 1) * GRP * P],
                                start=True,
                                stop=False,
                                skip_group_check=True,
                            )
                            for i in range(GRP):
                                t = grp * GRP + i
                                nc.tensor.matmul(
                                    out=grp_ps[:, i * P: (i + 1) * P],
                                    lhsT=triu[:],
                                    rhs=oh_sb[:, t * P: (t + 1) * P],
                                    start=False,
                                    stop=True,
                                    skip_group_check=True,
                                )
                            scratch = wpool.tile([P, GRP * P], FP32, tag="scr")
                            for i in range(GRP):
                                t = grp * GRP + i
                                nc.vector.scalar_tensor_tensor(
                                    out=scratch[:, i * P: (i + 1) * P],
                                    in0=grp_ps[:, i * P: (i + 1) * P],
                                    scalar=1.0,
                                    in1=oh_sb[:, t * P: (t + 1) * P],
                                    op0=mult,
                                    op1=mult,
                                    accum_out=flat_mat[:, t: t + 1],
                                )

                if MODE == "index":
                    nc.sync.dma_start(
                        out=out_dst[:, 0:1, 0:T], in_=flat_mat[:]
                    )
                    continue

                # --- idx shuffle: flat_mat [p, t] -> dma_gather layout ---
                xrep = xpool.tile([P, 8 * T], I16, tag="xrep")
                if LAYOUT == "pk":
                    # xrep[q', c*64 + k*8 + u] = flat_mat[8*(q'%16)+k, c*8+u]
                    assert CT == 8, "pk layout assumes 1024-token chunks"
                    NCH = T // CT  # chunks per batch
                    for h in range((NCH + 7) // 8):
                        ccs = list(range(h * 8, min((h + 1) * 8, NCH)))
                        rep_ps = ppool.tile([P, 512], FP32, tag="grp")
                        for ci, c in enumerate(ccs):
                            for k in range(8):
                                nc.tensor.matmul(
                                    out=rep_ps[:, ci * 64 + k * 8:
                                               ci * 64 + (k + 1) * 8],
                                    lhsT=rall2[:, k * P: (k + 1) * P],
                                    rhs=flat_mat[:, c * 8: (c + 1) * 8],
                                    start=True,
                                    stop=True,
                                )
                        nc.vector.tensor_copy(
                            xrep[:, h * 512: h * 512 + 64 * len(ccs)],
                            rep_ps[:, 0: 64 * len(ccs)],
                        )
                else:
                    for h in range(2):
                        rep_ps = ppool.tile([P, 4 * T], FP32, tag="grp")
                        for mm in range(4):
                            m = h * 4 + mm
                            nc.tensor.matmul(
                                out=rep_ps[:, mm * T: (mm + 1) * T],
                                lhsT=rall[:, m * P: (m + 1) * P],
                                rhs=flat_mat[:],
                                start=True,
                                stop=True,
                            )
                        # xrep[q', t*8 + h*4 + mm] = rep_ps[q', mm*T + t]
                        nc.vector.tensor_copy(
                            xrep[:].rearrange("p (t m) -> p t m", m=8)[:, :, h * 4: (h + 1) * 4],
                            rep_ps[:].rearrange("p (m t) -> p t m", t=T),
                        )

                # --- bulk gathers + stores ---
                for c in range(T // CT):
                    stg = spool.tile([P, CT * D], FP32, tag="stg")
                    if MODE == "sconly":
                        # Sequential HWDGE read + bulk scatter-add write.
                        nc.sync.dma_start(
                            out=stg[:].rearrange("p (c d) -> p c d", d=D),
                            in_=feat.ap()[b * FEAT_ROWS + c * NI:
                                          b * FEAT_ROWS + (c + 1) * NI]
                            .rearrange("(c p) d -> p c d", p=P),
                        )
                        nc.gpsimd.dma_scatter_add(
                            out.ap().rearrange("b n d -> (b n) d"),
                            stg[:].rearrange("p (c d) -> p c d", d=D),
                            xrep[:, c * CT * 8: (c + 1) * CT * 8],
                            NI,
                            NI,
                            D,
                            single_packet=bool(SP),
                        )
                        continue
                    nc.gpsimd.dma_gather(
                        stg[:].rearrange("p (t d) -> p t d", d=D),
                        feat.ap(),
                        xrep[:, c * CT * 8: (c + 1) * CT * 8],
                        NI,
                        NI,
                        D,
                        single_packet=bool(SP),
                        queue_num=c % QN,
                    )
                    if MODE != "gonly":
                        if LAYOUT == "pk":
                            nc.sync.dma_start(
                                out=out.ap()[b][c * NI: (c + 1) * NI, :]
                                .rearrange("(r k) d -> r k d", r=P),
                                in_=stg[:],
                            )
                        else:
                            nc.sync.dma_start(
                                out=out_dst[:, c * CT: (c + 1) * CT, :], in_=stg[:]
                            )

    nc.compile()
    return nc


_NC_CACHE = {}


def _get_nc():
    key = (NB, N_TOK, D_FEAT)
    if key not in _NC_CACHE:
        _NC_CACHE[key] = build_nc(*key)
    return _NC_CACHE[key]


def make_in_maps(block_features: np.ndarray, block_onehot: np.ndarray):
    """Shard full inputs batch-wise into 8 per-core input maps."""
    feat = np.ascontiguousarray(block_features, dtype=np.float32).reshape(
        B_FULL, N_TOK, D_FEAT
    )
    oh = np.ascontiguousarray(block_onehot, dtype=np.float32)
    nc = _get_nc()
    tag_shape = None
    for alloc in nc.m.functions[0].allocations:
        if isinstance(alloc, mybir.MemoryLocationSet) and alloc.kind == "ExternalInput":
            if alloc.memorylocations[0].name == "version_tag":
                tag_shape = tuple(alloc.tensor_shape)
    in_maps = []
    for c in range(N_CORES):
        lo, hi = c * NB, (c + 1) * NB
        m = {
            "block_features": feat[lo:hi].reshape(NB * N_TOK, D_FEAT),
            "block_onehot": oh[lo:hi],
        }
        if tag_shape is not None:
            m["version_tag"] = np.zeros(tag_shape, np.float32)
        in_maps.append(m)
    return in_maps


def run_spmd(in_maps, trace: bool = False):
    """Compile (cached) + run the SPMD program on cores 0-7."""
    nc = _get_nc()
    return bass_utils.run_bass_kernel_spmd(
        nc, in_maps, core_ids=list(range(N_CORES)), trace=trace
    )


def kernel(**inputs) -> np.ndarray:
    block_features = inputs["block_features"]
    block_onehot = inputs["block_onehot"]
    in_maps = make_in_maps(block_features, block_onehot)
    res = run_spmd(in_maps, trace=False)
    out = np.concatenate([r["out"] for r in res.results], axis=0)
    return out.reshape(B_FULL, N_TOK, D_FEAT)



# revision 7
# speedup vs baseline: 1.9273x; 1.9273x over previous
"""Trainium2 Bass kernel for nn_BlockUngrouper.

Problem: out[b, n, :] = block_features[b, g, k, :] where g is the block whose
one-hot claims token n and k is n's rank within that block (cumsum of the
one-hot along n).  The input distribution (per-sample permutation partition)
guarantees each token is claimed by exactly one block and ranks < 128, so per
batch this is a row-permutation gather with
    flat_idx[n] = 128 * g(n) + rank(n).

Sharding: data-parallel over the batch dim, 2 batches per NeuronCore x 8.

The gather is Q7-emission-bound (~9 ns per gathered row: the SWDGE firmware
emits one m2s + one s2m descriptor per element), so per core the 32K rows
floor at ~300 us.  v10 therefore restructures the per-batch program into a
chunk pipeline (16 token-tiles = 2048 tokens per chunk) so the first gather
launches after ~1/8 of the index work instead of all of it, and every later
chunk's index compute hides under the previous chunks' gathers:

  per chunk (16 tiles):
    1. onehot chunk -> SBUF f32 (HWDGE via ACT), ACT copy casts bf16.
    2. counts[g, t]: per tile, PE matmul lhsT=OH rhs=ones (PSUM cols).
    3. incl'[g, t]: DVE scan chained across chunks with initial=-1 (the -1
       converts inclusive counts to 0-based ranks); pex' = incl' - counts.
    4. PE-transpose pex' -> [t, g] bf16 (values in [-1,126]: bf16-exact),
       flatten 8-tile slices to a [2, 1024] bf16 row pair via tiny SBUF DMA
       (row 1 = static 128g + b*N, bf16-exact since it is 128*(g+128b)).
    5. per 4-tile PSUM group: K=2 bf16 matmul broadcasts (pex' + static)
       across partitions (start), 4 upper-triangular bf16 matmuls add the
       within-tile inclusive cumsum; DVE scalar_tensor_tensor (PSUM x onehot,
       accum_out over g) selects each token's entry -> flat_mat cols (exact
       fp32 PSUM arithmetic throughout).
    6. per 8-tile gather chunk: 8 selector matmuls reshape flat_mat into the
       dma_gather index layout (pk: slot j of a 1024-token chunk holds token
       8*(j%128) + j//128 so each partition's store run is 8KB contiguous),
       DVE copy casts PSUM->int16; ONE bulk SWDGE dma_gather pulls 1024 x 1KB
       feature rows into SBUF, then one HWDGE store writes them to out.
"""

from contextlib import ExitStack

import numpy as np

import concourse.bass as bass
import concourse.bacc as bacc
import concourse.mybir as mybir
import concourse.tile as tile
from concourse import bass_utils
from concourse.masks import make_identity, make_upper_triangular

P = 128  # partitions = tokens per tile = G (blocks) = NG_MAX
KERNEL_VERSION = 10  # bump on every meaningful kernel change (NEFF-cache buster)
N_CORES = 8
B_FULL = 16  # full batch dim
N_TOK = 16384  # tokens per batch
D_FEAT = 256  # feature dim
NB = B_FULL // N_CORES  # batches per core

FP32 = mybir.dt.float32
BF16 = mybir.dt.bfloat16
I16 = mybir.dt.int16
I32 = mybir.dt.int32


def build_nc(NB: int, N: int, D: int, CT: int = 8, GRP: int = 4, LC: int = 16,
             STG_BUFS: int = 4, IDX_BUFS: int = 4, REPS: int = 1,
             DYN_LOOP: int = 0, MODE: str = "full", SCRATCH: int = 65536,
             TIMING: bool = False, FEAT_ROWS: int = 0, SP: int = 1,
             QN: int = 1, LAYOUT: str = "pk"):
    """Build the per-core bass program.

    NB: batches per core; N: tokens per batch; D: feature dim.
    CT: tiles per gather/store chunk; GRP: tiles per PSUM group; LC: tiles
    per pipeline chunk (load/index granularity).
    TIMING: features/out become Internal DRAM tensors (no per-call host
    transfer; gather addresses depend only on onehot, values don't matter),
    and a tiny dummy output keeps the NEFF valid.  Timing-only builds.
    """
    T = N // P  # token tiles per batch
    assert T * P == N
    GRP = min(GRP, T)
    CT = min(CT, T)
    LC = min(LC, T)
    assert T % GRP == 0 and T % CT == 0 and T % LC == 0
    NI = CT * P  # tokens (gather indices) per chunk
    add = mybir.AluOpType.add
    mult = mybir.AluOpType.mult
    bypass = mybir.AluOpType.bypass

    nc = bacc.Bacc("TRN2", target_bir_lowering=False, debug=False,
                   dynamic_dma_scratch_size=SCRATCH, num_swdge_queues=QN)

    io_kind = "Internal" if TIMING else None
    FEAT_ROWS = FEAT_ROWS or N  # feat rows per batch (128 * G in general)
    feat = nc.dram_tensor("block_features", [NB * FEAT_ROWS, D], FP32,
                          kind=io_kind or "ExternalInput")
    oh = nc.dram_tensor("block_onehot", [NB, N, P], FP32, kind="ExternalInput")
    out = nc.dram_tensor("out", [NB, N, D], FP32, kind=io_kind or "ExternalOutput")
    if TIMING:
        nc.dram_tensor("dummy_out", [1, 8], FP32, kind="ExternalOutput")
    # The PJRT NEFF cache keys on the HLO alone (the embedded bass program
    # does not enter the hash), so distinct kernel versions collide.  A dummy
    # input whose shape encodes a version nonce forces a distinct hash.
    import zlib as _zlib
    _nonce = (
        _zlib.crc32(
            f"v{KERNEL_VERSION}-{NB}-{N}-{D}-{CT}-{GRP}-{LC}-{STG_BUFS}-{IDX_BUFS}-{REPS}-{DYN_LOOP}-{MODE}-{SCRATCH}-{TIMING}-{FEAT_ROWS}-{SP}-{QN}-{LAYOUT}".encode()
        )
        % 4093
        + 1
    )
    nc.dram_tensor("version_tag", [1, _nonce], FP32, kind="ExternalInput")

    with tile.TileContext(nc) as tc, ExitStack() as ctx:
        cpool = ctx.enter_context(tc.tile_pool(name="const", bufs=1))
        ohpool = ctx.enter_context(tc.tile_pool(name="ohp", bufs=3))
        ldpool = ctx.enter_context(tc.tile_pool(name="ld", bufs=3))
        wpool = ctx.enter_context(tc.tile_pool(name="work", bufs=3))
        tpool = ctx.enter_context(tc.tile_pool(name="tp", bufs=3))
        ppool = ctx.enter_context(tc.tile_pool(name="psum", bufs=2, space="PSUM"))
        pspool = ctx.enter_context(tc.tile_pool(name="psumsm", bufs=2, space="PSUM"))
        spool = ctx.enter_context(tc.tile_pool(name="stage", bufs=STG_BUFS))
        fpool = ctx.enter_context(tc.tile_pool(name="flat", bufs=4))
        xpool = ctx.enter_context(tc.tile_pool(name="xrep", bufs=IDX_BUFS))

        # --- constants ---
        triu = cpool.tile([P, P], BF16)  # triu[k, m] = 1 iff k <= m
        make_upper_triangular(nc, triu[:], val=1.0, diag=True)
        ident = cpool.tile([P, P], FP32)
        make_identity(nc, ident[:])
        ones_col = cpool.tile([P, 1], BF16)
        nc.gpsimd.memset(ones_col[:], 1.0)
        ones2 = cpool.tile([2, P], BF16)
        nc.gpsimd.memset(ones2[:], 1.0)
        scan_dummy = cpool.tile([P, LC], FP32)
        nc.gpsimd.memset(scan_dummy[:], 0.0)
        # gstat_b[0, t*128 + g] = 128*g + b*FEAT_ROWS  (bf16-exact: 128*(g+128b))
        gstats = []
        for b in range(NB):
            gi = cpool.tile([1, 8 * P], I32, tag=f"gsi{b}")
            nc.gpsimd.iota(gi[:], pattern=[[0, 8], [P, P]], base=b * FEAT_ROWS,
                           channel_multiplier=0)
            gf = cpool.tile([1, 8 * P], BF16, tag=f"gstat{b}")
            nc.vector.tensor_copy(gf[:], gi[:])
            gstats.append(gf)
        # Rall2[p, k*128 + q'] = 1 iff p == 8*(q'%16) + k  (pk-layout selector:
        # slot j of a 1024-token chunk maps to token base + 8*(j%128) + j//128,
        # so each partition's store run is 8 rows = 8KB contiguous)
        p_i = cpool.tile([P, 8 * P], I32)
        nc.gpsimd.iota(p_i[:], pattern=[[0, 8 * P]], base=0,
                       channel_multiplier=1)
        e2_i = cpool.tile([P, 8 * P], I32, tag="e2i")
        nc.gpsimd.iota(e2_i[:], pattern=[[1, 8], [0, 8], [8, 16]], base=0,
                       channel_multiplier=0)
        rall2 = cpool.tile([P, 8 * P], FP32, tag="rall2")
        nc.vector.tensor_tensor(out=rall2[:], in0=e2_i[:], in1=p_i[:],
                                op=mybir.AluOpType.is_equal)
        stg_c = None
        if MODE == "sonly":
            stg_c = cpool.tile([P, CT * D], FP32, tag="stgc")
            nc.gpsimd.memset(stg_c[:], 1.0)
        fiota = []
        if MODE in ("gather", "gonly", "sonly", "lonly", "glonly", "sconly", "gonly2k"):
            for b in range(NB):
                fi_i = cpool.tile([P, T], I32, tag=f"fi{b}")
                nc.gpsimd.iota(fi_i[:], pattern=[[P, T]], base=b * FEAT_ROWS,
                               channel_multiplier=1)
                ff = cpool.tile([P, T], FP32, tag=f"ff{b}")
                nc.vector.tensor_copy(ff[:], fi_i[:])
                fiota.append(ff)

        import contextlib
        loop_cm = tc.For_i(0, DYN_LOOP, 1) if DYN_LOOP else contextlib.nullcontext()
        with loop_cm:
          for rep in range(REPS):
            for b in range(NB):
                oh_src = oh.ap()[b].rearrange("(t p) g -> p t g", p=P)
                out_dst = out.ap()[b].rearrange("(t p) d -> p t d", p=P)

                if MODE == "gonly2k":
                    # dma_gather with 2KB elements (feat viewed [rows/2, 512]):
                    # same bytes, half the descriptors (iota only).
                    xr2 = xpool.tile([P, 4 * T], I16, tag="xr2")
                    fi2 = wpool.tile([P, 4 * T], I32, tag="fi2")
                    nc.gpsimd.iota(fi2[:], pattern=[[15, 4 * T]], base=b * FEAT_ROWS // 2,
                                   channel_multiplier=1)
                    nc.vector.tensor_copy(xr2[:], fi2[:])
                    for c in range(T // CT):
                        stg = spool.tile([P, CT * D], FP32, tag="stg")
                        nc.gpsimd.dma_gather(
                            stg[:].rearrange("p (t d) -> p t d", d=2 * D),
                            feat.ap().rearrange("(r two) d -> r (two d)", two=2),
                            xr2[:, c * CT * 4: (c + 1) * CT * 4],
                            NI // 2,
                            NI // 2,
                            2 * D,
                            single_packet=bool(SP),
                        )
                    continue
                if MODE == "glonly":
                    for c in range(T // CT):
                        stg = spool.tile([P, CT * D], FP32, tag="stg")
                        nc.gpsimd.dma_start(
                            out=stg[:].rearrange("p (t d) -> p t d", d=D),
                            in_=feat.ap()[b * FEAT_ROWS: (b + 1) * FEAT_ROWS]
                            .rearrange("(t p) d -> p t d", p=P)[:, c * CT: (c + 1) * CT, :],
                        )
                    continue
                if MODE == "lonly":
                    for c in range(T // CT):
                        stg = spool.tile([P, CT * D], FP32, tag="stg")
                        nc.sync.dma_start(
                            out=stg[:].rearrange("p (t d) -> p t d", d=D),
                            in_=feat.ap()[b * FEAT_ROWS: (b + 1) * FEAT_ROWS]
                            .rearrange("(t p) d -> p t d", p=P)[:, c * CT: (c + 1) * CT, :],
                        )
                    continue
                if MODE == "sonly":
                    for c in range(T // CT):
                        if LAYOUT == "pk":
                            nc.sync.dma_start(
                                out=out.ap()[b][c * NI: (c + 1) * NI, :]
                                .rearrange("(r k) d -> r k d", r=P),
                                in_=stg_c[:],
                            )
                        else:
                            nc.sync.dma_start(
                                out=out_dst[:, c * CT: (c + 1) * CT, :], in_=stg_c[:]
                            )
                    continue
                if MODE in ("gather", "gonly", "sconly"):
                    # Diagnostic: iota indices through the old full-batch idx
                    # shuffle, then gathers + stores only.
                    flat_full = fiota[b]
                    xrep = xpool.tile([P, 8 * T], I16, tag="xrepf")
                    NCH = T // CT
                    for h in range((NCH + 7) // 8):
                        ccs = list(range(h * 8, min((h + 1) * 8, NCH)))
                        rep_ps = ppool.tile([P, 512], FP32, tag="grp")
                        for ci, c in enumerate(ccs):
                            for k in range(8):
                                nc.tensor.matmul(
                                    out=rep_ps[:, ci * 64 + k * 8:
                                               ci * 64 + (k + 1) * 8],
                                    lhsT=rall2[:, k * P: (k + 1) * P],
                                    rhs=flat_full[:, c * 8: (c + 1) * 8],
                                    start=True,
                                    stop=True,
                                )
                        nc.vector.tensor_copy(
                            xrep[:, h * 512: h * 512 + 64 * len(ccs)],
                            rep_ps[:, 0: 64 * len(ccs)],
                        )
                    for c in range(T // CT):
                        stg = spool.tile([P, CT * D], FP32, tag="stg")
                        if MODE == "sconly":
                            nc.sync.dma_start(
                                out=stg[:].rearrange("p (c d) -> p c d", d=D),
                                in_=feat.ap()[b * FEAT_ROWS + c * NI:
                                              b * FEAT_ROWS + (c + 1) * NI]
                                .rearrange("(c p) d -> p c d", p=P),
                            )
                            nc.gpsimd.dma_scatter_add(
                                out.ap().rearrange("b n d -> (b n) d"),
                                stg[:].rearrange("p (c d) -> p c d", d=D),
                                xrep[:, c * CT * 8: (c + 1) * CT * 8],
                                NI,
                                NI,
                                D,
                                single_packet=bool(SP),
                            )
                            continue
                        nc.gpsimd.dma_gather(
                            stg[:].rearrange("p (t d) -> p t d", d=D),
                            feat.ap(),
                            xrep[:, c * CT * 8: (c + 1) * CT * 8],
                            NI,
                            NI,
                            D,
                            single_packet=bool(SP),
                            queue_num=c % QN,
                        )
                        if MODE != "gonly":
                            if LAYOUT == "pk":
                                nc.sync.dma_start(
                                    out=out.ap()[b][c * NI: (c + 1) * NI, :]
                                    .rearrange("(r k) d -> r k d", r=P),
                                    in_=stg[:],
                                )
                            else:
                                nc.sync.dma_start(
                                    out=out_dst[:, c * CT: (c + 1) * CT, :], in_=stg[:]
                                )
                    continue

                # ---- full / index: chunk-pipelined index compute + gather ----
                assert LC % CT == 0 and LC % GRP == 0
                SL = 8  # tiles per flatten slice (one [2, SL*P] bcast row pair)
                assert LC % SL == 0 and SL % GRP == 0
                incl_prev = None
                for lc in range(T // LC):
                    lc0 = lc * LC
                    # 1. load onehot chunk + cast bf16
                    ld = ldpool.tile([P, LC * P], FP32, tag="ld")
                    nc.scalar.dma_start(out=ld[:], in_=oh_src[:, lc0:lc0 + LC, :])
                    oh_sb = ohpool.tile([P, LC * P], BF16, tag="oh")
                    nc.scalar.copy(oh_sb[:], ld[:])
                    # 2. per-tile counts
                    counts_ps = pspool.tile([P, LC], FP32, tag="counts")
                    for t in range(LC):
                        nc.tensor.matmul(
                            out=counts_ps[:, t: t + 1],
                            lhsT=oh_sb[:, t * P: (t + 1) * P],
                            rhs=ones_col[:],
                            start=True,
                            stop=True,
                        )
                    # 3. chained scan (incl' = cumsum - 1), pex' = incl' - counts
                    incl = wpool.tile([P, LC], FP32, tag="incl")
                    nc.vector.tensor_tensor_scan(
                        out=incl[:],
                        data0=counts_ps[:],
                        data1=scan_dummy[:],
                        initial=(-1.0 if lc == 0 else incl_prev[:, LC - 1: LC]),
                        op0=add,
                        op1=bypass,
                    )
                    incl_prev = incl
                    pex = wpool.tile([P, LC], FP32, tag="pex")
                    nc.vector.tensor_tensor(
                        out=pex[:], in0=incl[:], in1=counts_ps[:],
                        op=mybir.AluOpType.subtract,
                    )
                    # 4. transpose to [t, g] bf16; flatten SL-slices + static row
                    padjT_ps = pspool.tile([LC, P], FP32, tag="padjT_ps")
                    nc.tensor.transpose(
                        out=padjT_ps[:], in_=pex[:], identity=ident[:]
                    )
                    padjT = tpool.tile([LC, P], BF16, tag="padjT")
                    nc.vector.tensor_copy(padjT[:], padjT_ps[:])
                    flat_mat = wpool.tile([P, LC], FP32, tag="flatmat")
                    for s in range(LC // SL):
                        flat_row = fpool.tile([2, SL * P], BF16, tag="flatrow")
                        nc.scalar.dma_start(
                            out=flat_row[0:1, :],
                            in_=padjT[s * SL: (s + 1) * SL, :],
                        )
                        nc.scalar.dma_start(
                            out=flat_row[1:2, :], in_=gstats[b][:]
                        )
                        # 5. groups: prefix bcast (K=2) + cumsum + select
                        for gi_ in range(SL // GRP):
                            grp_ps = ppool.tile([P, GRP * P], FP32, tag="grp")
                            nc.tensor.matmul(
                                out=grp_ps[:],
                                lhsT=ones2[:],
                                rhs=flat_row[:, gi_ * GRP * P: (gi_ + 1) * GRP * P],
                                start=True,
                                stop=False,
                                skip_group_check=True,
                            )
                            for i in range(GRP):
                                t = s * SL + gi_ * GRP + i  # tile within chunk
                                nc.tensor.matmul(
                                    out=grp_ps[:, i * P: (i + 1) * P],
                                    lhsT=triu[:],
                                    rhs=oh_sb[:, t * P: (t + 1) * P],
                                    start=False,
                                    stop=True,
                                    skip_group_check=True,
                                )
                            scratch = wpool.tile([P, GRP * P], FP32, tag="scr")
                            for i in range(GRP):
                                t = s * SL + gi_ * GRP + i
                                nc.vector.scalar_tensor_tensor(
                                    out=scratch[:, i * P: (i + 1) * P],
                                    in0=grp_ps[:, i * P: (i + 1) * P],
                                    scalar=1.0,
                                    in1=oh_sb[:, t * P: (t + 1) * P],
                                    op0=mult,
                                    op1=mult,
                                    accum_out=flat_mat[:, t: t + 1],
                                )

                    if MODE == "index":
                        nc.sync.dma_start(
                            out=out_dst[:, lc, 0:LC], in_=flat_mat[:]
                        )
                        continue

                    # 6. per CT-chunk: idx shuffle -> dma_gather -> store
                    for cc in range(LC // CT):
                        c = lc * (LC // CT) + cc  # global chunk id
                        rep_ps = pspool.tile([P, CT * 8], FP32, tag="rep")
                        for k in range(8):
                            nc.tensor.matmul(
                                out=rep_ps[:, k * 8: (k + 1) * 8],
                                lhsT=rall2[:, k * P: (k + 1) * P],
                                rhs=flat_mat[:, cc * CT: (cc + 1) * CT],
                                start=True,
                                stop=True,
                            )
                        xrep = xpool.tile([P, CT * 8], I16, tag="xrep")
                        nc.vector.tensor_copy(xrep[:], rep_ps[:])
                        stg = spool.tile([P, CT * D], FP32, tag="stg")
                        nc.gpsimd.dma_gather(
                            stg[:].rearrange("p (t d) -> p t d", d=D),
                            feat.ap(),
                            xrep[:],
                            NI,
                            NI,
                            D,
                            single_packet=bool(SP),
                            queue_num=c % QN,
                        )
                        if LAYOUT == "pk":
                            nc.sync.dma_start(
                                out=out.ap()[b][c * NI: (c + 1) * NI, :]
                                .rearrange("(r k) d -> r k d", r=P),
                                in_=stg[:],
                            )
                        else:
                            nc.sync.dma_start(
                                out=out_dst[:, c * CT: (c + 1) * CT, :], in_=stg[:]
                            )

    nc.compile()
    return nc


_NC_CACHE = {}


def _get_nc():
    key = (NB, N_TOK, D_FEAT)
    if key not in _NC_CACHE:
        _NC_CACHE[key] = build_nc(*key)
    return _NC_CACHE[key]


def make_in_maps(block_features: np.ndarray, block_onehot: np.ndarray):
    """Shard full inputs batch-wise into 8 per-core input maps."""
    feat = np.ascontiguousarray(block_features, dtype=np.float32).reshape(
        B_FULL, N_TOK, D_FEAT
    )
    oh = np.ascontiguousarray(block_onehot, dtype=np.float32)
    nc = _get_nc()
    tag_shape = None
    for alloc in nc.m.functions[0].allocations:
        if isinstance(alloc, mybir.MemoryLocationSet) and alloc.kind == "ExternalInput":
            if alloc.memorylocations[0].name == "version_tag":
                tag_shape = tuple(alloc.tensor_shape)
    in_maps = []
    for c in range(N_CORES):
        lo, hi = c * NB, (c + 1) * NB
        m = {
            "block_features": feat[lo:hi].reshape(NB * N_TOK, D_FEAT),
            "block_onehot": oh[lo:hi],
        }
        if tag_shape is not None:
            m["version_tag"] = np.zeros(tag_shape, np.float32)
        in_maps.append(m)
    return in_maps


def run_spmd(in_maps, trace: bool = False):
    """Compile (cached) + run the SPMD program on cores 0-7."""
    nc = _get_nc()
    return bass_utils.run_bass_kernel_spmd(
        nc, in_maps, core_ids=list(range(N_CORES)), trace=trace
    )


def kernel(**inputs) -> np.ndarray:
    block_features = inputs["block_features"]
    block_onehot = inputs["block_onehot"]
    in_maps = make_in_maps(block_features, block_onehot)
    res = run_spmd(in_maps, trace=False)
    out = np.concatenate([r["out"] for r in res.results], axis=0)
    return out.reshape(B_FULL, N_TOK, D_FEAT)


# revision 22
# speedup vs baseline: 2.2190x; 1.1513x over previous
"""Trainium2 Bass kernel for nn_BlockUngrouper.

Problem: out[b, n, :] = block_features[b, g, k, :] where g is the block whose
one-hot claims token n and k is n's rank within that block (cumsum of the
one-hot along n).  The input distribution (per-sample permutation partition)
guarantees each token is claimed by exactly one block and ranks < 128, so per
batch this is a row-permutation gather with
    flat_idx[n] = 128 * g(n) + rank(n).

Sharding: data-parallel over the batch dim, 2 batches per NeuronCore x 8.

The gather is Q7-emission-bound (~9 ns per gathered row: the SWDGE firmware
emits one m2s + one s2m descriptor per element), so per core the 32K rows
floor at ~300 us.  v10 therefore restructures the per-batch program into a
chunk pipeline (16 token-tiles = 2048 tokens per chunk) so the first gather
launches after ~1/8 of the index work instead of all of it, and every later
chunk's index compute hides under the previous chunks' gathers:

  per chunk (16 tiles):
    1. onehot chunk -> SBUF f32 (HWDGE via ACT), ACT copy casts bf16.
    2. counts[g, t]: per tile, PE matmul lhsT=OH rhs=ones (PSUM cols).
    3. incl'[g, t]: DVE scan chained across chunks with initial=-1 (the -1
       converts inclusive counts to 0-based ranks); pex' = incl' - counts.
    4. PE-transpose pex' -> [t, g] bf16 (values in [-1,126]: bf16-exact),
       flatten 8-tile slices to a [2, 1024] bf16 row pair via tiny SBUF DMA
       (row 1 = static 128g + b*N, bf16-exact since it is 128*(g+128b)).
    5. per 4-tile PSUM group: K=2 bf16 matmul broadcasts (pex' + static)
       across partitions (start), 4 upper-triangular bf16 matmuls add the
       within-tile inclusive cumsum; DVE scalar_tensor_tensor (PSUM x onehot,
       accum_out over g) selects each token's entry -> flat_mat cols (exact
       fp32 PSUM arithmetic throughout).
    6. per 8-tile gather chunk: 8 selector matmuls reshape flat_mat into the
       dma_gather index layout (pk: slot j of a 1024-token chunk holds token
       8*(j%128) + j//128 so each partition's store run is 8KB contiguous),
       DVE copy casts PSUM->int16; ONE bulk SWDGE dma_gather pulls 1024 x 1KB
       feature rows into SBUF, then one HWDGE store writes them to out.
"""

from contextlib import ExitStack

import numpy as np

import concourse.bass as bass
import concourse.bacc as bacc
import concourse.mybir as mybir
import concourse.tile as tile
from concourse import bass_utils
from concourse.masks import make_identity, make_upper_triangular

P = 128  # partitions = tokens per tile = G (blocks) = NG_MAX
KERNEL_VERSION = 10  # bump on every meaningful kernel change (NEFF-cache buster)
N_CORES = 8
B_FULL = 16  # full batch dim
N_TOK = 16384  # tokens per batch
D_FEAT = 256  # feature dim
NB = B_FULL // N_CORES  # batches per core

FP32 = mybir.dt.float32
BF16 = mybir.dt.bfloat16
I16 = mybir.dt.int16
I32 = mybir.dt.int32


def build_nc(NB: int, N: int, D: int, CT: int = 8, GRP: int = 4, LC: int = 16,
             STG_BUFS: int = 4, IDX_BUFS: int = 4, REPS: int = 1,
             DYN_LOOP: int = 0, MODE: str = "full", SCRATCH: int = 65536,
             TIMING: bool = False, FEAT_ROWS: int = 0, SP: int = 1,
             QN: int = 3, LAYOUT: str = "pk", GSPLIT: int = 4,
             LOADPAT: str = "tile", SENG: str = "sync"):
    """Build the per-core bass program.

    NB: batches per core; N: tokens per batch; D: feature dim.
    CT: tiles per gather/store chunk; GRP: tiles per PSUM group; LC: tiles
    per pipeline chunk (load/index granularity).
    TIMING: features/out become Internal DRAM tensors (no per-call host
    transfer; gather addresses depend only on onehot, values don't matter),
    and a tiny dummy output keeps the NEFF valid.  Timing-only builds.
    """
    T = N // P  # token tiles per batch
    assert T * P == N
    GRP = min(GRP, T)
    CT = min(CT, T)
    LC = min(LC, T)
    assert T % GRP == 0 and T % CT == 0 and T % LC == 0
    NI = CT * P  # tokens (gather indices) per chunk
    add = mybir.AluOpType.add
    mult = mybir.AluOpType.mult
    bypass = mybir.AluOpType.bypass

    nc = bacc.Bacc("TRN2", target_bir_lowering=False, debug=False,
                   dynamic_dma_scratch_size=SCRATCH, num_swdge_queues=QN)

    io_kind = "Internal" if TIMING else None
    FEAT_ROWS = FEAT_ROWS or N  # feat rows per batch (128 * G in general)
    feat = nc.dram_tensor("block_features", [NB * FEAT_ROWS, D], FP32,
                          kind=io_kind or "ExternalInput")
    oh = nc.dram_tensor("block_onehot", [NB, N, P], FP32, kind="ExternalInput")
    out = nc.dram_tensor("out", [NB, N, D], FP32, kind=io_kind or "ExternalOutput")
    if TIMING:
        nc.dram_tensor("dummy_out", [1, 8], FP32, kind="ExternalOutput")
    # The PJRT NEFF cache keys on the HLO alone (the embedded bass program
    # does not enter the hash), so distinct kernel versions collide.  A dummy
    # input whose shape encodes a version nonce forces a distinct hash.
    import zlib as _zlib
    _nonce = (
        _zlib.crc32(
            f"v{KERNEL_VERSION}-{NB}-{N}-{D}-{CT}-{GRP}-{LC}-{STG_BUFS}-{IDX_BUFS}-{REPS}-{DYN_LOOP}-{MODE}-{SCRATCH}-{TIMING}-{FEAT_ROWS}-{SP}-{QN}-{LAYOUT}-{GSPLIT}-{LOADPAT}-{SENG}".encode()
        )
        % 4093
        + 1
    )
    nc.dram_tensor("version_tag", [1, _nonce], FP32, kind="ExternalInput")

    with tile.TileContext(nc) as tc, ExitStack() as ctx:
        cpool = ctx.enter_context(tc.tile_pool(name="const", bufs=1))
        ohpool = ctx.enter_context(tc.tile_pool(name="ohp", bufs=3))
        ldpool = ctx.enter_context(tc.tile_pool(name="ld", bufs=3))
        wpool = ctx.enter_context(tc.tile_pool(name="work", bufs=3))
        tpool = ctx.enter_context(tc.tile_pool(name="tp", bufs=3))
        ppool = ctx.enter_context(tc.tile_pool(name="psum", bufs=2, space="PSUM"))
        pspool = ctx.enter_context(tc.tile_pool(name="psumsm", bufs=2, space="PSUM"))
        spool = ctx.enter_context(tc.tile_pool(name="stage", bufs=STG_BUFS))
        fpool = ctx.enter_context(tc.tile_pool(name="flat", bufs=4))
        xpool = ctx.enter_context(tc.tile_pool(name="xrep", bufs=IDX_BUFS))

        # --- constants ---
        triu = cpool.tile([P, P], BF16)  # triu[k, m] = 1 iff k <= m
        make_upper_triangular(nc, triu[:], val=1.0, diag=True)
        ident = cpool.tile([P, P], FP32)
        make_identity(nc, ident[:])
        ones_col = cpool.tile([P, 1], BF16)
        nc.gpsimd.memset(ones_col[:], 1.0)
        ones2 = cpool.tile([2, P], BF16)
        nc.gpsimd.memset(ones2[:], 1.0)
        scan_dummy = cpool.tile([P, LC], FP32)
        nc.gpsimd.memset(scan_dummy[:], 0.0)
        # gstat_b[0, t*128 + g] = 128*g + b*FEAT_ROWS  (bf16-exact: 128*(g+128b))
        gstats = []
        for b in range(NB):
            gi = cpool.tile([1, 8 * P], I32, tag=f"gsi{b}")
            nc.gpsimd.iota(gi[:], pattern=[[0, 8], [P, P]], base=b * FEAT_ROWS,
                           channel_multiplier=0)
            gf = cpool.tile([1, 8 * P], BF16, tag=f"gstat{b}")
            nc.vector.tensor_copy(gf[:], gi[:])
            gstats.append(gf)
        # Rall2[p, k*128 + q'] = 1 iff p == 8*(q'%16) + k  (pk-layout selector:
        # slot j of a 1024-token chunk maps to token base + 8*(j%128) + j//128,
        # so each partition's store run is 8 rows = 8KB contiguous)
        p_i = cpool.tile([P, 8 * P], I32)
        nc.gpsimd.iota(p_i[:], pattern=[[0, 8 * P]], base=0,
                       channel_multiplier=1)
        e2_i = cpool.tile([P, 8 * P], I32, tag="e2i")
        nc.gpsimd.iota(e2_i[:], pattern=[[1, 8], [0, 8], [8, 16]], base=0,
                       channel_multiplier=0)
        rall2 = cpool.tile([P, 8 * P], FP32, tag="rall2")
        nc.vector.tensor_tensor(out=rall2[:], in0=e2_i[:], in1=p_i[:],
                                op=mybir.AluOpType.is_equal)
        # Generalized pk selectors for a CT-tile gather chunk (NI = CT*128
        # tokens): element j -> partition j%128, col j//128; store wants
        # partition r to hold tokens r*CT..r*CT+CT-1 (CT KB contiguous), so
        # token(j) = (j%128)*CT + j//128.  With idx j at partition p=j%16
        # (s = p%16) and idx col = m*8 + u:
        #   flat row   q = CT*(s%SS) + m        (SS = 128//CT)
        #   flat col   t = c*CT + NS*u + s//SS  (NS = CT//8 splits)
        # The s//SS term means partition group h = s//SS reads a different
        # flat_mat column parity, so the shuffle accumulates NS selector
        # matmuls, split h masking partitions with (p%16)//SS == h.
        NS = CT // 8
        SS = 128 // CT
        rall_h = []
        if NS > 1:
            kt_i = cpool.tile([P, CT * P], I32, tag="kti")
            nc.gpsimd.iota(
                kt_i[:],
                pattern=[[1, CT], [0, 8], [0, NS], [CT, SS]],
                base=0,
                channel_multiplier=0,
            )
            h_i = cpool.tile([P, CT * P], I32, tag="hi")
            nc.gpsimd.iota(
                h_i[:],
                pattern=[[0, CT], [0, 8], [1, NS], [0, SS]],
                base=0,
                channel_multiplier=0,
            )
            pc_i = cpool.tile([P, CT * P], I32, tag="pci")
            nc.gpsimd.iota(pc_i[:], pattern=[[0, CT * P]], base=0,
                           channel_multiplier=1)
            keq = cpool.tile([P, CT * P], FP32, tag="keq")
            nc.vector.tensor_tensor(out=keq[:], in0=kt_i[:], in1=pc_i[:],
                                    op=mybir.AluOpType.is_equal)
            for h in range(NS):
                rh = cpool.tile([P, CT * P], FP32, tag=f"rall_h{h}")
                nc.vector.scalar_tensor_tensor(
                    out=rh[:], in0=h_i[:], scalar=float(h), in1=keq[:],
                    op0=mybir.AluOpType.is_equal, op1=mult,
                )
                rall_h.append(rh)
        stg_c = None
        if MODE == "sonly":
            stg_c = cpool.tile([P, CT * D], FP32, tag="stgc")
            nc.gpsimd.memset(stg_c[:], 1.0)
        fiota = []
        if MODE in ("gather", "gonly", "sonly", "lonly", "glonly", "sconly", "gonly2k"):
            for b in range(NB):
                fi_i = cpool.tile([P, T], I32, tag=f"fi{b}")
                nc.gpsimd.iota(fi_i[:], pattern=[[P, T]], base=b * FEAT_ROWS,
                               channel_multiplier=1)
                ff = cpool.tile([P, T], FP32, tag=f"ff{b}")
                nc.vector.tensor_copy(ff[:], fi_i[:])
                fiota.append(ff)

        import contextlib
        loop_cm = tc.For_i(0, DYN_LOOP, 1) if DYN_LOOP else contextlib.nullcontext()
        with loop_cm:
          for rep in range(REPS):
            for b in range(NB):
                oh_src = oh.ap()[b].rearrange("(t p) g -> p t g", p=P)
                out_dst = out.ap()[b].rearrange("(t p) d -> p t d", p=P)

                if MODE == "gonly2k":
                    # dma_gather with 2KB elements (feat viewed [rows/2, 512]):
                    # same bytes, half the descriptors (iota only).
                    xr2 = xpool.tile([P, 4 * T], I16, tag="xr2")
                    fi2 = wpool.tile([P, 4 * T], I32, tag="fi2")
                    nc.gpsimd.iota(fi2[:], pattern=[[15, 4 * T]], base=b * FEAT_ROWS // 2,
                                   channel_multiplier=1)
                    nc.vector.tensor_copy(xr2[:], fi2[:])
                    for c in range(T // CT):
                        stg = spool.tile([P, CT * D], FP32, tag="stg")
                        nc.gpsimd.dma_gather(
                            stg[:].rearrange("p (t d) -> p t d", d=2 * D),
                            feat.ap().rearrange("(r two) d -> r (two d)", two=2),
                            xr2[:, c * CT * 4: (c + 1) * CT * 4],
                            NI // 2,
                            NI // 2,
                            2 * D,
                            single_packet=bool(SP),
                        )
                    continue
                if MODE == "glonly":
                    for c in range(T // CT):
                        stg = spool.tile([P, CT * D], FP32, tag="stg")
                        nc.gpsimd.dma_start(
                            out=stg[:].rearrange("p (t d) -> p t d", d=D),
                            in_=feat.ap()[b * FEAT_ROWS: (b + 1) * FEAT_ROWS]
                            .rearrange("(t p) d -> p t d", p=P)[:, c * CT: (c + 1) * CT, :],
                        )
                    continue
                if MODE == "lonly":
                    for c in range(T // CT):
                        stg = spool.tile([P, CT * D], FP32, tag="stg")
                        nc.sync.dma_start(
                            out=stg[:].rearrange("p (t d) -> p t d", d=D),
                            in_=feat.ap()[b * FEAT_ROWS: (b + 1) * FEAT_ROWS]
                            .rearrange("(t p) d -> p t d", p=P)[:, c * CT: (c + 1) * CT, :],
                        )
                    continue
                if MODE == "sonly":
                    for c in range(T // CT):
                        if LAYOUT == "pk":
                            nc.sync.dma_start(
                                out=out.ap()[b][c * NI: (c + 1) * NI, :]
                                .rearrange("(r k) d -> r k d", r=P),
                                in_=stg_c[:],
                            )
                        else:
                            nc.sync.dma_start(
                                out=out_dst[:, c * CT: (c + 1) * CT, :], in_=stg_c[:]
                            )
                    continue
                if MODE in ("gather", "gonly", "sconly"):
                    # Diagnostic: iota indices through the old full-batch idx
                    # shuffle, then gathers + stores only.
                    flat_full = fiota[b]
                    xrep = xpool.tile([P, 8 * T], I16, tag="xrepf")
                    NCH = T // CT
                    for h in range((NCH + 7) // 8):
                        ccs = list(range(h * 8, min((h + 1) * 8, NCH)))
                        rep_ps = ppool.tile([P, 512], FP32, tag="grp")
                        for ci, c in enumerate(ccs):
                            for k in range(8):
                                nc.tensor.matmul(
                                    out=rep_ps[:, ci * 64 + k * 8:
                                               ci * 64 + (k + 1) * 8],
                                    lhsT=rall2[:, k * P: (k + 1) * P],
                                    rhs=flat_full[:, c * 8: (c + 1) * 8],
                                    start=True,
                                    stop=True,
                                )
                        nc.vector.tensor_copy(
                            xrep[:, h * 512: h * 512 + 64 * len(ccs)],
                            rep_ps[:, 0: 64 * len(ccs)],
                        )
                    for c in range(T // CT):
                        stg = spool.tile([P, CT * D], FP32, tag="stg")
                        if MODE == "sconly":
                            nc.sync.dma_start(
                                out=stg[:].rearrange("p (c d) -> p c d", d=D),
                                in_=feat.ap()[b * FEAT_ROWS + c * NI:
                                              b * FEAT_ROWS + (c + 1) * NI]
                                .rearrange("(c p) d -> p c d", p=P),
                            )
                            nc.gpsimd.dma_scatter_add(
                                out.ap().rearrange("b n d -> (b n) d"),
                                stg[:].rearrange("p (c d) -> p c d", d=D),
                                xrep[:, c * CT * 8: (c + 1) * CT * 8],
                                NI,
                                NI,
                                D,
                                single_packet=bool(SP),
                            )
                            continue
                        nc.gpsimd.dma_gather(
                            stg[:].rearrange("p (t d) -> p t d", d=D),
                            feat.ap(),
                            xrep[:, c * CT * 8: (c + 1) * CT * 8],
                            NI,
                            NI,
                            D,
                            single_packet=bool(SP),
                            queue_num=c % QN,
                        )
                        if MODE != "gonly":
                            if LAYOUT == "pk":
                                nc.sync.dma_start(
                                    out=out.ap()[b][c * NI: (c + 1) * NI, :]
                                    .rearrange("(r k) d -> r k d", r=P),
                                    in_=stg[:],
                                )
                            else:
                                nc.sync.dma_start(
                                    out=out_dst[:, c * CT: (c + 1) * CT, :], in_=stg[:]
                                )
                    continue

                # ---- full / index: chunk-pipelined index compute + gather ----
                assert LC % GRP == 0 and (LC % CT == 0 or CT % LC == 0)
                SL = 8  # tiles per flatten slice (one [2, SL*P] bcast row pair)
                assert LC % SL == 0 and SL % GRP == 0
                incl_prev = None
                flat_mat = wpool.tile([P, T], FP32, tag="flatmat")
                for lc in range(T // LC):
                    lc0 = lc * LC
                    # 1. load onehot chunk + cast bf16
                    ld = ldpool.tile([P, LC * P], FP32, tag="ld")
                    if LOADPAT == "contig" and TIMING:
                        # Timing-only probe: contiguous 8KB descriptors (wrong
                        # SBUF layout but still one-hot rows, so all the index
                        # arithmetic stays in range).
                        nc.scalar.dma_start(
                            out=ld[:],
                            in_=oh.ap()[b].rearrange("(l p q) g -> l p (q g)",
                                                     p=P, q=LC)[lc],
                        )
                    else:
                        nc.scalar.dma_start(out=ld[:], in_=oh_src[:, lc0:lc0 + LC, :])
                    oh_sb = ohpool.tile([P, LC * P], BF16, tag="oh")
                    nc.scalar.copy(oh_sb[:], ld[:])
                    # 2. per-tile counts
                    counts_ps = pspool.tile([P, LC], FP32, tag="counts")
                    for t in range(LC):
                        nc.tensor.matmul(
                            out=counts_ps[:, t: t + 1],
                            lhsT=oh_sb[:, t * P: (t + 1) * P],
                            rhs=ones_col[:],
                            start=True,
                            stop=True,
                        )
                    # 3. chained scan (incl' = cumsum - 1), pex' = incl' - counts
                    incl = wpool.tile([P, LC], FP32, tag="incl")
                    nc.vector.tensor_tensor_scan(
                        out=incl[:],
                        data0=counts_ps[:],
                        data1=scan_dummy[:],
                        initial=(-1.0 if lc == 0 else incl_prev[:, LC - 1: LC]),
                        op0=add,
                        op1=bypass,
                    )
                    incl_prev = incl
                    pex = wpool.tile([P, LC], FP32, tag="pex")
                    nc.vector.tensor_tensor(
                        out=pex[:], in0=incl[:], in1=counts_ps[:],
                        op=mybir.AluOpType.subtract,
                    )
                    # 4. transpose to [t, g] bf16; flatten SL-slices + static row
                    padjT_ps = pspool.tile([LC, P], FP32, tag="padjT_ps")
                    nc.tensor.transpose(
                        out=padjT_ps[:], in_=pex[:], identity=ident[:]
                    )
                    padjT = tpool.tile([LC, P], BF16, tag="padjT")
                    nc.vector.tensor_copy(padjT[:], padjT_ps[:])
                    for s in range(LC // SL):
                        flat_row = fpool.tile([2, SL * P], BF16, tag="flatrow")
                        nc.scalar.dma_start(
                            out=flat_row[0:1, :],
                            in_=padjT[s * SL: (s + 1) * SL, :],
                        )
                        nc.scalar.dma_start(
                            out=flat_row[1:2, :], in_=gstats[b][:]
                        )
                        # 5. groups: prefix bcast (K=2) + cumsum + select
                        for gi_ in range(SL // GRP):
                            grp_ps = ppool.tile([P, GRP * P], FP32, tag="grp")
                            nc.tensor.matmul(
                                out=grp_ps[:],
                                lhsT=ones2[:],
                                rhs=flat_row[:, gi_ * GRP * P: (gi_ + 1) * GRP * P],
                                start=True,
                                stop=False,
                                skip_group_check=True,
                            )
                            for i in range(GRP):
                                t = s * SL + gi_ * GRP + i  # tile within chunk
                                nc.tensor.matmul(
                                    out=grp_ps[:, i * P: (i + 1) * P],
                                    lhsT=triu[:],
                                    rhs=oh_sb[:, t * P: (t + 1) * P],
                                    start=False,
                                    stop=True,
                                    skip_group_check=True,
                                )
                            scratch = wpool.tile([P, GRP * P], FP32, tag="scr")
                            for i in range(GRP):
                                t = s * SL + gi_ * GRP + i
                                nc.vector.scalar_tensor_tensor(
                                    out=scratch[:, i * P: (i + 1) * P],
                                    in0=grp_ps[:, i * P: (i + 1) * P],
                                    scalar=1.0,
                                    in1=oh_sb[:, t * P: (t + 1) * P],
                                    op0=mult,
                                    op1=mult,
                                    accum_out=flat_mat[:, lc0 + t: lc0 + t + 1],
                                )

                    if MODE == "index":
                        nc.sync.dma_start(
                            out=out_dst[:, lc, 0:LC], in_=flat_mat[:, lc0:lc0 + LC]
                        )
                        continue

                    # 6. per CT-chunk (once its flat_mat cols are complete):
                    #    idx shuffle -> dma_gather -> store
                    ready = [
                        c for c in range(T // CT)
                        if lc * LC < (c + 1) * CT <= (lc + 1) * LC
                    ]
                    for c in ready:
                        rep_ps = pspool.tile([P, CT * 8], FP32, tag="rep")
                        fslice = flat_mat[:, c * CT: (c + 1) * CT]
                        if NS == 1:
                            for k in range(8):
                                nc.tensor.matmul(
                                    out=rep_ps[:, k * 8: (k + 1) * 8],
                                    lhsT=rall2[:, k * P: (k + 1) * P],
                                    rhs=fslice,
                                    start=True,
                                    stop=True,
                                )
                        else:
                            fview = fslice.rearrange("p (u h) -> p h u", h=NS)
                            for m in range(CT):
                                for h in range(NS):
                                    nc.tensor.matmul(
                                        out=rep_ps[:, m * 8: (m + 1) * 8],
                                        lhsT=rall_h[h][:, m * P: (m + 1) * P],
                                        rhs=fview[:, h, :],
                                        start=(h == 0),
                                        stop=(h == NS - 1),
                                        skip_group_check=True,
                                    )
                        xrep = xpool.tile([P, CT * 8], I16, tag="xrep")
                        nc.vector.tensor_copy(xrep[:], rep_ps[:])
                        stg = spool.tile([P, CT * D], FP32, tag="stg")
                        NH = NI // GSPLIT  # idxs per gather op
                        CH = CT // GSPLIT  # stage cols per gather op
                        for gsp in range(GSPLIT):
                            nc.gpsimd.dma_gather(
                                stg[:, gsp * CH * D: (gsp + 1) * CH * D]
                                .rearrange("p (t d) -> p t d", d=D),
                                feat.ap(),
                                xrep[:, gsp * CH * 8: (gsp + 1) * CH * 8],
                                NH,
                                NH,
                                D,
                                single_packet=bool(SP),
                                queue_num=(c * GSPLIT + gsp) % QN,
                            )
                        seng = {"sync": nc.sync, "scalar": nc.scalar,
                                "vector": nc.vector}[SENG]
                        if LAYOUT == "pk":
                            seng.dma_start(
                                out=out.ap()[b][c * NI: (c + 1) * NI, :]
                                .rearrange("(r k) d -> r k d", r=P),
                                in_=stg[:],
                            )
                        else:
                            seng.dma_start(
                                out=out_dst[:, c * CT: (c + 1) * CT, :], in_=stg[:]
                            )

    nc.compile()
    return nc


_NC_CACHE = {}


def _get_nc():
    key = (NB, N_TOK, D_FEAT)
    if key not in _NC_CACHE:
        _NC_CACHE[key] = build_nc(*key)
    return _NC_CACHE[key]


def make_in_maps(block_features: np.ndarray, block_onehot: np.ndarray):
    """Shard full inputs batch-wise into 8 per-core input maps."""
    feat = np.ascontiguousarray(block_features, dtype=np.float32).reshape(
        B_FULL, N_TOK, D_FEAT
    )
    oh = np.ascontiguousarray(block_onehot, dtype=np.float32)
    nc = _get_nc()
    tag_shape = None
    for alloc in nc.m.functions[0].allocations:
        if isinstance(alloc, mybir.MemoryLocationSet) and alloc.kind == "ExternalInput":
            if alloc.memorylocations[0].name == "version_tag":
                tag_shape = tuple(alloc.tensor_shape)
    in_maps = []
    for c in range(N_CORES):
        lo, hi = c * NB, (c + 1) * NB
        m = {
            "block_features": feat[lo:hi].reshape(NB * N_TOK, D_FEAT),
            "block_onehot": oh[lo:hi],
        }
        if tag_shape is not None:
            m["version_tag"] = np.zeros(tag_shape, np.float32)
        in_maps.append(m)
    return in_maps


def run_spmd(in_maps, trace: bool = False):
    """Compile (cached) + run the SPMD program on cores 0-7."""
    nc = _get_nc()
    return bass_utils.run_bass_kernel_spmd(
        nc, in_maps, core_ids=list(range(N_CORES)), trace=trace
    )


def kernel(**inputs) -> np.ndarray:
    block_features = inputs["block_features"]
    block_onehot = inputs["block_onehot"]
    in_maps = make_in_maps(block_features, block_onehot)
    res = run_spmd(in_maps, trace=False)
    out = np.concatenate([r["out"] for r in res.results], axis=0)
    return out.reshape(B_FULL, N_TOK, D_FEAT)
